# revision 22
# baseline (speedup 1.0000x reference)
"""Self-contained Trainium2 Bass kernel for causal attention with relative
position bias (B=4, T=1024, D=1024, H=16, dh=64), SPMD across 8 NeuronCores.

Sharding: core = (batch b = core//2, head-half g = core%2). Each core computes
QKV projections for its 8 heads, causal attention, and a partial output
projection; partials are summed pairwise with chunked on-device ReduceScatter.

v2 design: query-major AV output. Per (head, 128-query block qb) the AV matmul
computes psum[128 q, 65] = sum_{jb<=qb} pt[:,jb,qb-block]^T @ V_aug[jb] with a
ones column giving the softmax denominator as a per-partition column. The
normalization is then a DVE reciprocal + per-partition tensor_scalar multiply
(no DRAM round trips, no partition broadcasts), followed by a PE transpose of
the [128 q, 128 c] head-pair block back to channel-major for the O-projection.

Layouts (per core):
  xT    [128, 8, 1024]  bf16   x[b].T as [d%128, d//128, t]
  wq/wk [128, 8, 512]   bf16   W[:, g*512:+512] as [d%128, d//128, n]
  wv    [128, 8, 512]   bf16   same
  wo    [128, 4, 1024]  bf16   Wo[g*512:+512, :] as [n%128, n//128, m]
  dbias [128, 8, 1024]  f8e5   per local head: bias[j%128, i-128jb] with
                               causal mask folded in as -57344 (exp -> 0)
  QT/KT [128, 4, 1024]  bf16   [n%128, n//128, t]  (channel-major)
  V_aug [128, 8, 8, 65] bf16   [t%128, t//128, h, c] with ones column c=64
  pt    [128, 2, 8, 1024] bf16 exp((logits+bias)/64), [j%128, h%2, jb, i]
  atq   [128, 2, 4, 8, 64] bf16 normalized AV out, [q%128, phase, qb, h, c]
  at    [128, 4, 1024]  bf16   channel-major normalized attention (O-proj in)
"""
import sys

sys.path.insert(0, "/opt/trn_rl_repo")

import os

import numpy as np
import ml_dtypes

B, T, D = 4, 1024, 1024
H, DH = 16, 64
HL, NL = 8, 512  # local heads / channels per core
NCORES = 8
NEG = -1.0e9

_CACHE = {}


PH_LO = [0, 512, 768]
PH_HI = [512, 768, 1024]
PH_QBS = [[0, 1, 2, 3], [4, 5], [6, 7]]


def _phase_tiles(p):
    """Logit tiles (jb, i0, w) for phase p; tile k has jb=k."""
    tiles = []
    for jb in range(PH_HI[p] // 128):
        i0 = max(PH_LO[p], 128 * jb)
        tiles.append((jb, i0, PH_HI[p] - i0))
    return tiles


def _build():
    from concourse import bass
    from contextlib import ExitStack

    mybir = bass.mybir
    f32, bf16 = mybir.dt.float32, mybir.dt.bfloat16
    f8 = mybir.dt.float8e5

    nc = bass.Bass(target_bir_lowering=False, debug=False)
    xT = nc.declare_dram_parameter("xT", [128, 8, T], bf16, isOutput=False)
    wq = nc.declare_dram_parameter("wq", [128, 8, NL], bf16, isOutput=False)
    wk = nc.declare_dram_parameter("wk", [128, 8, NL], bf16, isOutput=False)
    wv = nc.declare_dram_parameter("wv", [128, 8, NL], bf16, isOutput=False)
    wo = nc.declare_dram_parameter("wo", [128, 4, D], bf16, isOutput=False)
    dbias = nc.declare_dram_parameter("dbias", [128, HL, T], bf16, isOutput=False)
    idb = nc.declare_dram_parameter("idb", [128, 128], bf16, isOutput=False)
    bo_rep = nc.declare_dram_parameter("bo_rep", [128, D], f32, isOutput=False)
    out = nc.declare_dram_parameter("out", [T // 2, D], bf16, isOutput=True)

    partials = [nc.dram_tensor("partial0", [T // 2, D], bf16),
                nc.dram_tensor("partial1", [T // 4, D], bf16),
                nc.dram_tensor("partial2", [T // 4, D], bf16)]
    red = nc.dram_tensor("red", [T // 2, D], bf16)

    ctx = ExitStack()
    sem = lambda n: ctx.enter_context(nc.semaphore(n))
    sb = lambda n, shape, dt: ctx.enter_context(nc.sbuf_tensor(n, shape, dt))
    ps = lambda n, shape, dt: ctx.enter_context(nc.psum_tensor(n, shape, dt))

    s_wq = sem("s_wq")
    s_x0a = sem("s_x0a")
    s_x0b = sem("s_x0b")
    s_wk = sem("s_wk")
    s_x1 = sem("s_x1")
    s_wv = sem("s_wv")
    s_d = sem("s_d")
    s_wo = sem("s_wo")
    s_pe = sem("s_pe")
    s_dve = sem("s_dve")
    s_act = sem("s_act")
    s_out = sem("s_out")
    s_cc = sem("s_cc")
    s_fin = sem("s_fin")

    xT_sb = sb("xT_sb", [128, 8, T], bf16)
    wq_sb = sb("wq_sb", [128, 8, NL], bf16)
    wk_sb = sb("wk_sb", [128, 8, NL], bf16)
    wv_sb = sb("wv_sb", [128, 8, NL], bf16)
    wo_sb = sb("wo_sb", [128, 4, D], bf16)
    qz_sb = sb("qz_sb", [128, 8, T], bf16)
    kt_sb = sb("kt_sb", [128, 4, T], bf16)
    va_sb = sb("va_sb", [128, 8, HL, 65], bf16)
    pt_sb = sb("pt_sb", [128, 2, 8, T], bf16)
    db_all = sb("db_all", [128, HL, T], bf16)
    idb_sb = sb("idb_sb", [128, 128], bf16)
    bo_sb = sb("bo_sb", [128, D], f32)
    atq_sb = sb("atq_sb", [128, 8, NL], bf16)
    avc_sb = sb("avc_sb", [128, 4, DH], f32)
    rs_sb = sb("rs_sb", [128, 8, HL], f32)
    at_sb = sb("at_sb", [128, 4, T], bf16)
    stg_own = sb("stg_own", [128, 16, 512], bf16)

    ps_mm = [ps("ps_mm0", [128, 512], f32), ps("ps_mm1", [128, 512], f32)]
    ps_lg = [ps(f"ps_lg{i}", [128, 512], f32) for i in range(3)]
    ps_av = [ps("ps_av0", [128, 512], f32), ps("ps_av1", [128, 512], f32)]
    ps_tr = ps("ps_tr", [128, 4, 128], bf16)

    # ---- plan ----
    ops = {k: [] for k in ("sp", "pe", "dve", "act", "gp")}

    def wait(eng, s, v):
        ops[eng].append(("wait", s, v))

    def op(eng, fn, inc=None):
        ops[eng].append(("op", fn, inc))

    cnt = {"pe": 0, "dve": 0, "act": 0}
    rec = {}

    # --- input DMAs (SP), ordered for earliest PE start ---
    op("sp", lambda e: e.dma_start(out=wq_sb[:], in_=wq[:]), (s_wq, 16))
    op("sp", lambda e: e.dma_start(out=xT_sb[:, 0:4, 0:512], in_=xT[:, 0:4, 0:512]), (s_x0a, 16))
    op("sp", lambda e: e.dma_start(out=xT_sb[:, 4:8, 0:512], in_=xT[:, 4:8, 0:512]), (s_x0b, 16))
    op("sp", lambda e: e.dma_start(out=wk_sb[:], in_=wk[:]), (s_wk, 16))
    op("sp", lambda e: e.dma_start(out=xT_sb[:, :, 512:1024], in_=xT[:, :, 512:1024]), (s_x1, 16))
    op("sp", lambda e: e.dma_start(out=wv_sb[:], in_=wv[:]), (s_wv, 16))
    op("sp", lambda e: e.dma_start(out=db_all[:], in_=dbias[:, :, :]), (s_d, 16))
    op("sp", lambda e: e.dma_start(out=idb_sb[:], in_=idb[:, :]), (s_d, 16))
    op("sp", lambda e: e.dma_start(out=bo_sb[:], in_=bo_rep[:]), (s_d, 16))
    op("sp", lambda e: e.dma_start(out=wo_sb[:], in_=wo[:]), (s_wo, 16))

    # --- DVE: V ones column (needed by first AV) ---
    op("dve", lambda e: e.memset(va_sb[:, :, :, 64:65], 1.0), (s_dve, 1))
    cnt["dve"] += 1
    # zero the dead half of each per-head padded-q slab so QK can contract
    # over all 128 partitions at full PE rate (64-row tiles run at half rate)
    for h in range(HL):
        z0 = 64 * (1 - (h % 2))
        op("dve", (lambda h=h, z0=z0: lambda e: e.memset(
            qz_sb[z0:z0 + 64, h, :], 0.0))(), (s_dve, 1))
        cnt["dve"] += 1

    # --- HAM warm-up: dummy matmuls on untouched SBUF during the input-DMA
    # wait (ps_lg0 is cleared by its first real start=True use). ---
    for _w in range(8):
        op("pe", (lambda: lambda e: e.matmul(
            ps_lg[0][:, :], stg_own[:, 0, 0:128], stg_own[:, 1, :],
            start=True, stop=True))(), None)

    # --- QKV projections: 24 psum groups of 8 matmuls ---
    # order: q-tc0 (nb0..3), k-tc0, q-tc1, k-tc1, v (tb0..7)
    qkv = []
    for nb in range(4):
        qkv.append(("q", nb, 0))
    for nb in range(4):
        qkv.append(("k", nb, 0))
    for nb in range(4):
        qkv.append(("q", nb, 1))
    for nb in range(4):
        qkv.append(("k", nb, 1))
    for tb in range(8):
        qkv.append(("v", tb, None))

    for g, item in enumerate(qkv):
        slot = ps_mm[g % 2]
        kind = item[0]
        if g == 0:
            wait("pe", s_wq, 16)
            wait("pe", s_x0a, 16)
        elif g == 1:
            wait("pe", s_x0b, 16)
        elif g == 4:
            wait("pe", s_wk, 16)
        elif g == 8:
            wait("pe", s_x1, 16)
        elif g == 16:
            wait("pe", s_wv, 16)
        if g >= 2:
            ceng, cval = rec[("cp", g - 2)]
            wait("pe", s_act if ceng == "act" else s_dve, cval)
        for db in range(8):
            st, sp_ = db == 0, db == 7
            if g == 0 and db == 4:
                wait("pe", s_x0b, 16)
            if kind == "q" or kind == "k":
                _, nb, tc = item
                w = wq_sb if kind == "q" else wk_sb
                fn = (lambda w=w, nb=nb, tc=tc, db=db, slot=slot, st=st, sp_=sp_: lambda e: e.matmul(
                    slot[:, :], w[:, db, nb * 128:(nb + 1) * 128], xT_sb[:, db, tc * 512:(tc + 1) * 512],
                    start=st, stop=sp_))()
            else:
                _, tb, _n = item
                fn = (lambda tb=tb, db=db, slot=slot, st=st, sp_=sp_: lambda e: e.matmul(
                    slot[:, :], xT_sb[:, db, tb * 128:(tb + 1) * 128], wv_sb[:, db, 0:NL],
                    start=st, stop=sp_))()
            op("pe", fn, (s_pe, 1) if sp_ else None)
        cnt["pe"] += 1
        rec[("mm", g)] = cnt["pe"]

        # evacuate psum: q/k copies on ACT (idle until exps), v copies on DVE
        if kind == "q":
            _, nb, tc = item
            wait("dve", s_pe, rec[("mm", g)])
            fn = (lambda nb=nb, tc=tc, slot=slot: lambda e: e.tensor_copy(
                qz_sb[0:64, 2 * nb, tc * 512:(tc + 1) * 512], slot[0:64, :]))()
            op("dve", fn, (s_dve, 1))
            cnt["dve"] += 1
            fn = (lambda nb=nb, tc=tc, slot=slot: lambda e: e.tensor_copy(
                qz_sb[64:128, 2 * nb + 1, tc * 512:(tc + 1) * 512], slot[64:128, :]))()
            op("dve", fn, (s_dve, 1))
            cnt["dve"] += 1
            rec[("cp", g)] = ("dve", cnt["dve"])
        elif kind == "k":
            _, nb, tc = item
            wait("dve", s_pe, rec[("mm", g)])
            fn = (lambda nb=nb, tc=tc, slot=slot: lambda e: e.tensor_copy(
                kt_sb[:, nb, tc * 512:(tc + 1) * 512], slot[:, :]))()
            op("dve", fn, (s_dve, 1))
            cnt["dve"] += 1
            rec[("cp", g)] = ("dve", cnt["dve"])
        else:
            _, tb, _n = item
            wait("dve", s_pe, rec[("mm", g)])
            fn = (lambda tb=tb, slot=slot: lambda e: e.tensor_copy(
                va_sb[:, tb, :, 0:64], slot[:, :]))()
            op("dve", fn, (s_dve, 1))
            cnt["dve"] += 1
            rec[("cp", g)] = ("dve", cnt["dve"])
    n_qkcp = 16  # ACT ops before first exp

    # --- attention ---
    LL = [0]   # global logit tile counter (for ps_lg slot rotation)
    AVC = [0]  # global AV counter (ps_av slot rotation)
    TRC = [0]  # global transpose counter (ps_tr slot rotation)
    slot_last = [None] * 3  # act-count of last exp reading each lg slot

    def plan_lg(p, h, k):
        jb, i0, w = _phase_tiles(p)[k]
        par = h % 2
        g2 = h % 2
        nbh = h // 2
        si = LL[0] % 3
        slot = ps_lg[si]
        if slot_last[si] is not None:
            wait("pe", s_act, slot_last[si])
        if LL[0] == 0:
            wait("pe", s_d, 32)  # dbias + id8 loaded
        # qt/kt availability: phase-0 needs tc0 copies; phase-1 tiles touch tc1
        if p == 0:
            wait("pe", s_dve, rec[("cp", 4 + nbh)][1])  # k-tc0 copy (q earlier)
        else:
            wait("pe", s_dve, rec[("cp", 12 + nbh)][1])  # k-tc1 copy
        u0 = i0 - 128 * jb
        fn = (lambda h=h, nbh=nbh, jb=jb, i0=i0, w=w, slot=slot: lambda e: e.matmul(
            slot[:, 0:w],
            kt_sb[:, nbh, 128 * jb:128 * jb + 128],
            qz_sb[:, h, i0:i0 + w],
            start=True, stop=False))()
        op("pe", fn, None)
        fn = (lambda h=h, u0=u0, w=w, slot=slot: lambda e: e.matmul(
            slot[:, 0:w], idb_sb[:, :], db_all[:, h, u0:u0 + w],
            start=False, stop=True))()
        op("pe", fn, (s_pe, 1))
        cnt["pe"] += 1
        rec[("lg", (p, h, jb))] = cnt["pe"]

        wait("act", s_pe, rec[("lg", (p, h, jb))])
        fn = (lambda par=par, jb=jb, i0=i0, w=w, slot=slot: lambda e: e.activation(
            pt_sb[:, par, jb, i0:i0 + w], slot[:, 0:w],
            bass.mybir.ActivationFunctionType.Exp, scale=1.0 / 64.0))()
        op("act", fn, (s_act, 1))
        cnt["act"] += 1
        slot_last[si] = cnt["act"]
        rec[("exp", (p, h, jb))] = cnt["act"]
        LL[0] += 1

    def plan_av(p, h, qb):
        par = h % 2
        si = AVC[0] % 2
        wait("pe", s_act, rec[("exp", (p, h, qb))])
        veng, vval = rec[("cp", 16 + qb)]
        wait("pe", s_dve, vval)  # v copies tb<=qb done (DVE in-order)
        if AVC[0] >= 2:
            # bank si is written next; its previous reader chain (AVC-2) must
            # be done — PSUM banks fault on concurrent PE-write + engine-read
            wait("pe", s_dve, rec[("norm", AVC[0] - 2)])
        for jb in range(qb + 1):
            st, sp_ = jb == 0, jb == qb
            fn = (lambda par=par, jb=jb, qb=qb, h=h, si=si, st=st, sp_=sp_: lambda e: e.matmul(
                ps_av[si][:, 0:65],
                pt_sb[:, par, jb, 128 * qb:128 * qb + 128],
                va_sb[:, jb, h, 0:65],
                start=st, stop=sp_))()
            op("pe", fn, (s_pe, 1) if sp_ else None)
        cnt["pe"] += 1
        rec[("av", AVC[0])] = cnt["pe"]
        rec[("avref", (p, h, qb))] = AVC[0]

        wait("dve", s_pe, rec[("av", AVC[0])])
        fn = (lambda qb=qb, h=h, si=si: lambda e: e.reciprocal(
            rs_sb[:, qb, h:h + 1], ps_av[si][:, 64:65]))()
        op("dve", fn, (s_dve, 1))
        cnt["dve"] += 1
        # stage the AV psum to SBUF, then normalize with a per-partition
        # scalar entirely on DVE (tensor_scalar with PSUM in0 + AP scalar is
        # broken on HW; the SBUF-in0 form is exact)
        fn = (lambda si=si, sj=AVC[0] % 4: lambda e: e.tensor_copy(
            avc_sb[:, sj, :], ps_av[si][:, 0:64]))()
        op("dve", fn, (s_dve, 1))
        cnt["dve"] += 1
        rec[("norm", AVC[0])] = cnt["dve"]  # frees the ps_av slot
        fn = (lambda qb=qb, h=h, sj=AVC[0] % 4: lambda e: e.tensor_scalar_mul(
            atq_sb[:, qb, 64 * h:64 * h + 64], avc_sb[:, sj, :],
            rs_sb[:, qb, h:h + 1]))()
        op("dve", fn, (s_dve, 1))
        cnt["dve"] += 1
        rec[("normref", (p, h, qb))] = cnt["dve"]
        AVC[0] += 1

    def plan_tr(p, hb, qb):
        si = TRC[0] % 4
        wait("pe", s_dve, rec[("normref", (p, 2 * hb + 1, qb))])
        if TRC[0] >= 1:
            wait("pe", s_dve, rec[("atcp", TRC[0] - 1)])
        if TRC[0] == 0:
            wait("pe", s_d, 32)  # idb loaded
        fn = (lambda qb=qb, hb=hb, si=si: lambda e: e.matmul(
            ps_tr[:, si, :], atq_sb[:, qb, 128 * hb:128 * hb + 128], idb_sb[:, :],
            is_transpose=True))()
        op("pe", fn, (s_pe, 1))
        cnt["pe"] += 1
        rec[("tr", TRC[0])] = cnt["pe"]

        wait("dve", s_pe, rec[("tr", TRC[0])])
        fn = (lambda hb=hb, qb=qb, si=si: lambda e: e.tensor_copy(
            at_sb[:, hb, 128 * qb:128 * qb + 128], ps_tr[:, si, :]))()
        op("dve", fn, (s_dve, 1))
        cnt["dve"] += 1
        rec[("atcp", TRC[0])] = cnt["dve"]
        rec[("atcp2", (hb, qb))] = cnt["dve"]
        TRC[0] += 1

    def plan_head(p, h, skip_tiles=0):
        """Emit this head's logit tiles + AVs, interleaving the previous
        pair's transposes (on even h >= 2)."""
        trs = [("tr", h // 2 - 1, q) for q in PH_QBS[p]] if (h % 2 == 0 and h >= 2) else []
        stream = []
        if p == 0:
            order = ["lg0", "lg1", "av0", "lg2", "av1", "lg3", "av2", "av3"]
        elif p == 1:
            order = ["lg0", "lg1", "lg2", "lg3", "lg4", "lg5", "av4", "av5"]
        else:
            order = ["lg0", "lg1", "lg2", "lg3", "lg4", "lg5", "lg6", "lg7",
                     "av6", "av7"]
        ti = 0
        for it in order:
            kind, idx = it[:2], int(it[2])
            if kind == "lg" and idx < skip_tiles:
                continue
            stream.append((kind, idx))
            if ti < len(trs) and kind == "lg":
                stream.append(trs[ti])
                ti += 1
        for t in trs[ti:]:
            stream.append(t)
        for item in stream:
            if item[0] == "lg":
                plan_lg(p, h, item[1])
            elif item[0] == "av":
                plan_av(p, h, item[1])
            else:
                plan_tr(p, item[1], item[2])

    def plan_oproj(p, interleave_lg=None):
        # trailing pair (hb3) transposes first
        for q in PH_QBS[p]:
            plan_tr(p, 3, q)
        ilg = list(interleave_lg or [])
        nj = 2 * len(PH_QBS[p])
        jj0 = [0, 8, 12][p]
        for j in range(nj):
            tb, mc = PH_QBS[p][0] + j // 2, j % 2
            jj = jj0 + j
            slot = ps_mm[j % 2]
            if jj == 0:
                wait("pe", s_wo, 16)
            if j >= 2:
                wait("pe", s_dve, rec[("stage", jj - 2)])
            elif p == 0:
                ceng, cval = rec[("cp", 22 + j)]
                wait("pe", s_dve, cval)  # last v copies freed ps_mm
            else:
                wait("pe", s_dve, rec[("stage", jj - 2)])  # prev phase stages
            for nb in range(4):
                st, sp_ = nb == 0, nb == 3
                if nb == 3 or j < 2:
                    wait("pe", s_dve, rec[("atcp2", (nb, tb))])
                fn = (lambda nb=nb, tb=tb, mc=mc, slot=slot, st=st, sp_=sp_: lambda e: e.matmul(
                    slot[:, :], at_sb[:, nb, tb * 128:(tb + 1) * 128], wo_sb[:, nb, mc * 512:(mc + 1) * 512],
                    start=st, stop=sp_))()
                op("pe", fn, (s_pe, 1) if sp_ else None)
            cnt["pe"] += 1
            rec[("op", jj)] = cnt["pe"]
            if ilg:
                h2, k2, p2 = ilg.pop(0)
                plan_lg(p2, h2, k2)

            wait("dve", s_pe, rec[("op", jj)])
            if jj == 0:
                wait("dve", s_d, 48)  # bo loaded
            fn = (lambda jj=jj, mc=mc, slot=slot: lambda e: e.tensor_add(
                stg_own[:, jj, :], slot[:, :], bo_sb[:, mc * 512:(mc + 1) * 512]))()
            op("dve", fn, (s_dve, 1))
            cnt["dve"] += 1
            rec[("stage", jj)] = cnt["dve"]

            wait("sp", s_dve, rec[("stage", jj)])
            pdst = partials[p]
            fn = (lambda j=j, jj=jj, pdst=pdst: lambda e: e.dma_start(
                out=pdst[(j // 2) * 128:(j // 2 + 1) * 128, (j % 2) * 512:(j % 2 + 1) * 512],
                in_=stg_own[:, jj, :]))()
            op("sp", fn, (s_out, 16))

        # fire this phase's ReduceScatter in 256-row chunks
        for c_ in range(nj // 4):
            q_ = jj0 // 4 + c_
            wait("gp", s_out, 16 * (jj0 + 4 * (c_ + 1)))
            op("gp", (lambda p=p, c_=c_, q_=q_: lambda e: e.collective_compute(
                "ReduceScatter", bass.mybir.AluOpType.add,
                replica_groups=[[0, 1], [2, 3], [4, 5], [6, 7]],
                ins=[partials[p][256 * c_:256 * (c_ + 1), :]],
                outs=[red[128 * q_:128 * (q_ + 1), :]]))(), (s_cc, 1))

    stage = os.environ.get("V2STAGE", "full")
    if stage == "lg":
        # logits+exp only, all 8 heads of phase 0
        for h in range(HL):
            for k in range(4):
                plan_lg(0, h, k)
        wait("gp", s_act, cnt["act"])
        for nb in range(4):
            op("gp", (lambda nb=nb: lambda e: e.dma_start(
                out=out[128 * nb:128 * (nb + 1), :], in_=pt_sb[:, 0, nb, :]))(), (s_fin, 16))
        wait("gp", s_fin, 64)
    elif stage == "noav":
        # logits + AV + recip/norm, no transposes
        for h in range(HL):
            if p0 := True:
                for it in ["lg0", "lg1", "av0", "lg2", "av1", "lg3", "av2", "av3"]:
                    kind, idx = it[:2], int(it[2])
                    if kind == "lg":
                        plan_lg(0, h, idx)
                    else:
                        plan_av(0, h, idx)
        wait("gp", s_act, cnt["act"])
        for qbl in range(4):
            op("gp", (lambda qbl=qbl: lambda e: e.dma_start(
                out=out[128 * qbl:128 * (qbl + 1), 0:512],
                in_=atq_sb[:, 0, qbl, :]))(), (s_fin, 16))
        wait("gp", s_fin, 64)
    elif stage == "qkv":
        # truncated: dump qt to out
        wait("gp", s_dve, rec[("cp", 23)][1])
        for nb in range(4):
            op("gp", (lambda nb=nb: lambda e: e.dma_start(
                out=out[128 * nb:128 * (nb + 1), :], in_=qt_sb[:, nb, :]))(), (s_fin, 16))
        wait("gp", s_fin, 64)
    elif stage == "ph0":
        for h in range(HL):
            plan_head(0, h)
        for q in range(4):
            plan_tr(0, 3, q)
        wait("gp", s_dve, rec[("atcp", TRC[0] - 1)])
        for nb in range(4):
            op("gp", (lambda nb=nb: lambda e: e.dma_start(
                out=out[128 * nb:128 * (nb + 1), :], in_=at_sb[:, nb, :]))(), (s_fin, 16))
        wait("gp", s_fin, 64)
    elif stage == "op0":
        for h in range(HL):
            plan_head(0, h)
        plan_oproj(0)
        for q_ in range(2):
            wait("gp", s_cc, q_ + 1)
            op("gp", (lambda q_=q_: lambda e: e.dma_start(
                out=out[128 * q_:128 * (q_ + 1), :],
                in_=red[128 * q_:128 * (q_ + 1), :]))(), (s_fin, 16))
        wait("gp", s_fin, 32)
    else:
        for h in range(HL):
            plan_head(0, h)
        plan_oproj(0, interleave_lg=[(0, 0, 1), (0, 1, 1), (0, 2, 1), (0, 3, 1)])
        for h in range(HL):
            plan_head(1, h, skip_tiles=4 if h == 0 else 0)
        plan_oproj(1, interleave_lg=[(0, 0, 2), (0, 1, 2), (0, 2, 2), (0, 3, 2)])
        for h in range(HL):
            plan_head(2, h, skip_tiles=4 if h == 0 else 0)
        plan_oproj(2)

        # --- copy RS chunks to out as they land ---
        for q_ in range(4):
            wait("gp", s_cc, q_ + 1)
            op("gp", (lambda q_=q_: lambda e: e.dma_start(
                out=out[128 * q_:128 * (q_ + 1), :],
                in_=red[128 * q_:128 * (q_ + 1), :]))(), (s_fin, 16))
        wait("gp", s_fin, 64)

    _build.ops_debug = {k: list(v) for k, v in ops.items()}

    # ---- emit ----
    def emit(eng, lst):
        for item in lst:
            if item[0] == "wait":
                eng.wait_ge(item[1], item[2])
            else:
                inst = item[1](eng)
                if item[2] is not None:
                    inst.then_inc(item[2][0], item[2][1])

    with nc.Block() as block:
        @block.sync
        def _(e):
            emit(e, ops["sp"])

        @block.tensor
        def _(e):
            emit(e, ops["pe"])

        @block.vector
        def _(e):
            emit(e, ops["dve"])

        @block.scalar
        def _(e):
            emit(e, ops["act"])

        @block.gpsimd
        def _(e):
            emit(e, ops["gp"])

    ctx.close()
    return nc


def _get_nc():
    if "nc" not in _CACHE:
        _CACHE["nc"] = _build()
    return _CACHE["nc"]


def _prep_inputs(x, Wq, Wk, Wv, Wo, bo, rel_pos_bias):
    bf = ml_dtypes.bfloat16
    in_maps = []
    p_idx = np.arange(128)[:, None]
    u_idx = np.arange(T)[None, :]
    for core in range(NCORES):
        b, g = core // 2, core % 2
        xb = np.asarray(x[b], dtype=np.float32)
        xT_h = np.ascontiguousarray(
            xb.T.reshape(8, 128, T).transpose(1, 0, 2)).astype(bf)
        wq_h = np.ascontiguousarray(
            Wq[:, g * NL:(g + 1) * NL].reshape(8, 128, NL).transpose(1, 0, 2)).astype(bf)
        wk_h = np.ascontiguousarray(
            Wk[:, g * NL:(g + 1) * NL].reshape(8, 128, NL).transpose(1, 0, 2)).astype(bf)
        wv_h = np.ascontiguousarray(
            Wv[:, g * NL:(g + 1) * NL].reshape(8, 128, NL).transpose(1, 0, 2)).astype(bf)
        wo_h = np.ascontiguousarray(
            Wo[g * NL:(g + 1) * NL, :].reshape(4, 128, D).transpose(1, 0, 2)).astype(bf)
        db = np.empty((128, HL, T), dtype=bf)
        dif = np.clip(u_idx - p_idx, 0, T - 1)
        msk = u_idx >= p_idx
        for h in range(HL):
            rev = np.asarray(rel_pos_bias[g * HL + h], dtype=np.float32)[::-1]
            db[:, h, :] = np.where(msk, rev[dif], NEG).astype(bf)
        bo_h = np.broadcast_to(np.asarray(bo, np.float32) * 0.5, (128, D)).copy()
        in_maps.append({
            "xT": xT_h, "wq": wq_h, "wk": wk_h, "wv": wv_h, "wo": wo_h,
            "dbias": db, "bo_rep": bo_h, "idb": np.eye(128, dtype=bf),
        })
    return in_maps


def run_on_device(x, Wq, Wk, Wv, Wo, bo, rel_pos_bias, trace=False):
    from concourse.bass_utils import run_bass_kernel_spmd

    nc = _get_nc()
    in_maps = _prep_inputs(x, Wq, Wk, Wv, Wo, bo, rel_pos_bias)
    res = run_bass_kernel_spmd(nc, in_maps, core_ids=list(range(NCORES)), trace=trace)
    outs = []
    for b in range(B):
        ev = res.results[2 * b]["out"]
        od = res.results[2 * b + 1]["out"]
        g = np.empty((T, D), dtype=ev.dtype)
        for blk in range(4):
            g[256 * blk:256 * blk + 128] = ev[128 * blk:128 * blk + 128]
            g[256 * blk + 128:256 * blk + 256] = od[128 * blk:128 * blk + 128]
        outs.append(g)
    out = np.stack(outs).astype(np.float32)
    return out, res


def kernel(x, Wq, Wk, Wv, Wo, bo, rel_pos_bias):
    out, _ = run_on_device(x, Wq, Wk, Wv, Wo, bo, rel_pos_bias, trace=False)
    return out


# revision 23
# speedup vs baseline: 1.0370x; 1.0370x over previous
"""Self-contained Trainium2 Bass kernel for causal attention with relative
position bias (B=4, T=1024, D=1024, H=16, dh=64), SPMD across 8 NeuronCores.

Sharding: core = (batch b = core//2, head-half g = core%2). Each core computes
QKV projections for its 8 heads, causal attention, and a partial output
projection; partials are summed pairwise with chunked on-device ReduceScatter.

v2 design: query-major AV output. Per (head, 128-query block qb) the AV matmul
computes psum[128 q, 65] = sum_{jb<=qb} pt[:,jb,qb-block]^T @ V_aug[jb] with a
ones column giving the softmax denominator as a per-partition column. The
normalization is then a DVE reciprocal + per-partition tensor_scalar multiply
(no DRAM round trips, no partition broadcasts), followed by a PE transpose of
the [128 q, 128 c] head-pair block back to channel-major for the O-projection.

Layouts (per core):
  xT    [128, 8, 1024]  bf16   x[b].T as [d%128, d//128, t]
  wq/wk [128, 8, 512]   bf16   W[:, g*512:+512] as [d%128, d//128, n]
  wv    [128, 8, 512]   bf16   same
  wo    [128, 4, 1024]  bf16   Wo[g*512:+512, :] as [n%128, n//128, m]
  dbias [128, 8, 1024]  f8e5   per local head: bias[j%128, i-128jb] with
                               causal mask folded in as -57344 (exp -> 0)
  QT/KT [128, 4, 1024]  bf16   [n%128, n//128, t]  (channel-major)
  V_aug [128, 8, 8, 65] bf16   [t%128, t//128, h, c] with ones column c=64
  pt    [128, 2, 8, 1024] bf16 exp((logits+bias)/64), [j%128, h%2, jb, i]
  atq   [128, 2, 4, 8, 64] bf16 normalized AV out, [q%128, phase, qb, h, c]
  at    [128, 4, 1024]  bf16   channel-major normalized attention (O-proj in)
"""
import sys

sys.path.insert(0, "/opt/trn_rl_repo")

import os

import numpy as np
import ml_dtypes

B, T, D = 4, 1024, 1024
H, DH = 16, 64
HL, NL = 8, 512  # local heads / channels per core
NCORES = 8
NEG = -1.0e9

_CACHE = {}


def _phase_tiles(p):
    """Logit tiles (jb, i0, w) for phase p; tile k has jb=k."""
    tiles = []
    if p == 0:
        for jb in range(4):
            tiles.append((jb, 128 * jb, 512 - 128 * jb))
    else:
        for jb in range(8):
            i0 = max(512, 128 * jb)
            tiles.append((jb, i0, 1024 - i0))
    return tiles


def _build():
    from concourse import bass
    from contextlib import ExitStack

    mybir = bass.mybir
    f32, bf16 = mybir.dt.float32, mybir.dt.bfloat16
    f8 = mybir.dt.float8e5

    nc = bass.Bass(target_bir_lowering=False, debug=False)
    xT = nc.declare_dram_parameter("xT", [128, 8, T], bf16, isOutput=False)
    wq = nc.declare_dram_parameter("wq", [128, 8, NL], bf16, isOutput=False)
    wk = nc.declare_dram_parameter("wk", [128, 8, NL], bf16, isOutput=False)
    wv = nc.declare_dram_parameter("wv", [128, 8, NL], bf16, isOutput=False)
    wo = nc.declare_dram_parameter("wo", [128, 4, D], bf16, isOutput=False)
    dbias = nc.declare_dram_parameter("dbias", [128, HL, T], bf16, isOutput=False)
    idb = nc.declare_dram_parameter("idb", [128, 128], bf16, isOutput=False)
    bo_rep = nc.declare_dram_parameter("bo_rep", [128, D], f32, isOutput=False)
    out = nc.declare_dram_parameter("out", [T // 2, D], bf16, isOutput=True)

    partials = [nc.dram_tensor(f"partial{i}", [T // 2, D], bf16) for i in range(2)]
    red = nc.dram_tensor("red", [T // 2, D], bf16)

    ctx = ExitStack()
    sem = lambda n: ctx.enter_context(nc.semaphore(n))
    sb = lambda n, shape, dt: ctx.enter_context(nc.sbuf_tensor(n, shape, dt))
    ps = lambda n, shape, dt: ctx.enter_context(nc.psum_tensor(n, shape, dt))

    s_wq = sem("s_wq")
    s_x0a = sem("s_x0a")
    s_x0b = sem("s_x0b")
    s_wk = sem("s_wk")
    s_x1 = sem("s_x1")
    s_wv = sem("s_wv")
    s_d = sem("s_d")
    s_wo = sem("s_wo")
    s_pe = sem("s_pe")
    s_dve = sem("s_dve")
    s_act = sem("s_act")
    s_out = sem("s_out")
    s_cc = sem("s_cc")
    s_fin = sem("s_fin")

    xT_sb = sb("xT_sb", [128, 8, T], bf16)
    wq_sb = sb("wq_sb", [128, 8, NL], bf16)
    wk_sb = sb("wk_sb", [128, 8, NL], bf16)
    wv_sb = sb("wv_sb", [128, 8, NL], bf16)
    wo_sb = sb("wo_sb", [128, 4, D], bf16)
    qz_sb = sb("qz_sb", [128, 8, T], bf16)
    kt_sb = sb("kt_sb", [128, 4, T], bf16)
    va_sb = sb("va_sb", [128, 8, HL, 65], bf16)
    pt_sb = sb("pt_sb", [128, 2, 8, T], bf16)
    db_all = sb("db_all", [128, HL, T], bf16)
    idb_sb = sb("idb_sb", [128, 128], bf16)
    bo_sb = sb("bo_sb", [128, D], f32)
    atq_sb = sb("atq_sb", [128, 2, 4, NL], bf16)
    avc_sb = sb("avc_sb", [128, 4, DH], f32)
    rs_sb = sb("rs_sb", [128, 2, 4, HL], f32)
    at_sb = sb("at_sb", [128, 4, T], bf16)
    stg_own = sb("stg_own", [128, 16, 512], bf16)

    ps_mm = [ps("ps_mm0", [128, 512], f32), ps("ps_mm1", [128, 512], f32)]
    ps_lg = [ps(f"ps_lg{i}", [128, 512], f32) for i in range(3)]
    ps_av = [ps("ps_av0", [128, 512], f32), ps("ps_av1", [128, 512], f32)]
    ps_tr = ps("ps_tr", [128, 4, 128], bf16)

    # ---- plan ----
    ops = {k: [] for k in ("sp", "pe", "dve", "act", "gp")}

    def wait(eng, s, v):
        ops[eng].append(("wait", s, v))

    def op(eng, fn, inc=None):
        ops[eng].append(("op", fn, inc))

    cnt = {"pe": 0, "dve": 0, "act": 0}
    rec = {}

    # --- input DMAs (SP), ordered for earliest PE start ---
    op("sp", lambda e: e.dma_start(out=wq_sb[:], in_=wq[:]), (s_wq, 16))
    op("sp", lambda e: e.dma_start(out=xT_sb[:, 0:4, 0:512], in_=xT[:, 0:4, 0:512]), (s_x0a, 16))
    op("sp", lambda e: e.dma_start(out=xT_sb[:, 4:8, 0:512], in_=xT[:, 4:8, 0:512]), (s_x0b, 16))
    op("sp", lambda e: e.dma_start(out=wk_sb[:], in_=wk[:]), (s_wk, 16))
    op("sp", lambda e: e.dma_start(out=xT_sb[:, :, 512:1024], in_=xT[:, :, 512:1024]), (s_x1, 16))
    op("sp", lambda e: e.dma_start(out=wv_sb[:], in_=wv[:]), (s_wv, 16))
    op("sp", lambda e: e.dma_start(out=db_all[:], in_=dbias[:, :, :]), (s_d, 16))
    op("sp", lambda e: e.dma_start(out=idb_sb[:], in_=idb[:, :]), (s_d, 16))
    op("sp", lambda e: e.dma_start(out=bo_sb[:], in_=bo_rep[:]), (s_d, 16))
    op("sp", lambda e: e.dma_start(out=wo_sb[:], in_=wo[:]), (s_wo, 16))

    # --- DVE: V ones column (needed by first AV) ---
    op("dve", lambda e: e.memset(va_sb[:, :, :, 64:65], 1.0), (s_dve, 1))
    cnt["dve"] += 1
    # zero the dead half of each per-head padded-q slab so QK can contract
    # over all 128 partitions at full PE rate (64-row tiles run at half rate)
    for h in range(HL):
        z0 = 64 * (1 - (h % 2))
        op("dve", (lambda h=h, z0=z0: lambda e: e.memset(
            qz_sb[z0:z0 + 64, h, :], 0.0))(), (s_dve, 1))
        cnt["dve"] += 1

    # --- HAM warm-up: dummy matmuls on untouched SBUF during the input-DMA
    # wait (ps_lg0 is cleared by its first real start=True use). ---
    for _w in range(8):
        op("pe", (lambda: lambda e: e.matmul(
            ps_lg[0][:, :], stg_own[:, 0, 0:128], stg_own[:, 1, :],
            start=True, stop=True))(), None)

    # --- QKV projections: 24 psum groups of 8 matmuls ---
    # order: q-tc0 (nb0..3), k-tc0, q-tc1, k-tc1, v (tb0..7)
    qkv = []
    for nb in range(4):
        qkv.append(("q", nb, 0))
    for nb in range(4):
        qkv.append(("k", nb, 0))
    for nb in range(4):
        qkv.append(("q", nb, 1))
    for nb in range(4):
        qkv.append(("k", nb, 1))
    for tb in range(8):
        qkv.append(("v", tb, None))

    for g, item in enumerate(qkv):
        slot = ps_mm[g % 2]
        kind = item[0]
        if g == 0:
            wait("pe", s_wq, 16)
            wait("pe", s_x0a, 16)
        elif g == 1:
            wait("pe", s_x0b, 16)
        elif g == 4:
            wait("pe", s_wk, 16)
        elif g == 8:
            wait("pe", s_x1, 16)
        elif g == 16:
            wait("pe", s_wv, 16)
        if g >= 2:
            ceng, cval = rec[("cp", g - 2)]
            wait("pe", s_act if ceng == "act" else s_dve, cval)
        for db in range(8):
            st, sp_ = db == 0, db == 7
            if g == 0 and db == 4:
                wait("pe", s_x0b, 16)
            if kind == "q" or kind == "k":
                _, nb, tc = item
                w = wq_sb if kind == "q" else wk_sb
                fn = (lambda w=w, nb=nb, tc=tc, db=db, slot=slot, st=st, sp_=sp_: lambda e: e.matmul(
                    slot[:, :], w[:, db, nb * 128:(nb + 1) * 128], xT_sb[:, db, tc * 512:(tc + 1) * 512],
                    start=st, stop=sp_))()
            else:
                _, tb, _n = item
                fn = (lambda tb=tb, db=db, slot=slot, st=st, sp_=sp_: lambda e: e.matmul(
                    slot[:, :], xT_sb[:, db, tb * 128:(tb + 1) * 128], wv_sb[:, db, 0:NL],
                    start=st, stop=sp_))()
            op("pe", fn, (s_pe, 1) if sp_ else None)
        cnt["pe"] += 1
        rec[("mm", g)] = cnt["pe"]

        # evacuate psum: q/k copies on ACT (idle until exps), v copies on DVE
        if kind == "q":
            _, nb, tc = item
            wait("dve", s_pe, rec[("mm", g)])
            fn = (lambda nb=nb, tc=tc, slot=slot: lambda e: e.tensor_copy(
                qz_sb[0:64, 2 * nb, tc * 512:(tc + 1) * 512], slot[0:64, :]))()
            op("dve", fn, (s_dve, 1))
            cnt["dve"] += 1
            fn = (lambda nb=nb, tc=tc, slot=slot: lambda e: e.tensor_copy(
                qz_sb[64:128, 2 * nb + 1, tc * 512:(tc + 1) * 512], slot[64:128, :]))()
            op("dve", fn, (s_dve, 1))
            cnt["dve"] += 1
            rec[("cp", g)] = ("dve", cnt["dve"])
        elif kind == "k":
            _, nb, tc = item
            wait("dve", s_pe, rec[("mm", g)])
            fn = (lambda nb=nb, tc=tc, slot=slot: lambda e: e.tensor_copy(
                kt_sb[:, nb, tc * 512:(tc + 1) * 512], slot[:, :]))()
            op("dve", fn, (s_dve, 1))
            cnt["dve"] += 1
            rec[("cp", g)] = ("dve", cnt["dve"])
        else:
            _, tb, _n = item
            wait("dve", s_pe, rec[("mm", g)])
            fn = (lambda tb=tb, slot=slot: lambda e: e.tensor_copy(
                va_sb[:, tb, :, 0:64], slot[:, :]))()
            op("dve", fn, (s_dve, 1))
            cnt["dve"] += 1
            rec[("cp", g)] = ("dve", cnt["dve"])
    n_qkcp = 16  # ACT ops before first exp

    # --- attention ---
    LL = [0]   # global logit tile counter (for ps_lg slot rotation)
    AVC = [0]  # global AV counter (ps_av slot rotation)
    TRC = [0]  # global transpose counter (ps_tr slot rotation)
    slot_last = [None] * 3  # act-count of last exp reading each lg slot

    def plan_lg(p, h, k):
        jb, i0, w = _phase_tiles(p)[k]
        par = h % 2
        g2 = h % 2
        nbh = h // 2
        si = LL[0] % 3
        slot = ps_lg[si]
        if slot_last[si] is not None:
            wait("pe", s_act, slot_last[si])
        if LL[0] == 0:
            wait("pe", s_d, 32)  # dbias + id8 loaded
        # qt/kt availability: phase-0 needs tc0 copies; phase-1 tiles touch tc1
        if p == 0:
            wait("pe", s_dve, rec[("cp", 4 + nbh)][1])  # k-tc0 copy (q earlier)
        else:
            wait("pe", s_dve, rec[("cp", 12 + nbh)][1])  # k-tc1 copy
        u0 = i0 - 128 * jb
        fn = (lambda h=h, nbh=nbh, jb=jb, i0=i0, w=w, slot=slot: lambda e: e.matmul(
            slot[:, 0:w],
            kt_sb[:, nbh, 128 * jb:128 * jb + 128],
            qz_sb[:, h, i0:i0 + w],
            start=True, stop=False))()
        op("pe", fn, None)
        fn = (lambda h=h, u0=u0, w=w, slot=slot: lambda e: e.matmul(
            slot[:, 0:w], idb_sb[:, :], db_all[:, h, u0:u0 + w],
            start=False, stop=True))()
        op("pe", fn, (s_pe, 1))
        cnt["pe"] += 1
        rec[("lg", (p, h, jb))] = cnt["pe"]

        wait("act", s_pe, rec[("lg", (p, h, jb))])
        fn = (lambda par=par, jb=jb, i0=i0, w=w, slot=slot: lambda e: e.activation(
            pt_sb[:, par, jb, i0:i0 + w], slot[:, 0:w],
            bass.mybir.ActivationFunctionType.Exp, scale=1.0 / 64.0))()
        op("act", fn, (s_act, 1))
        cnt["act"] += 1
        slot_last[si] = cnt["act"]
        rec[("exp", (p, h, jb))] = cnt["act"]
        LL[0] += 1

    def plan_av(p, h, qb):
        # qb is the GLOBAL query block (0..7); phase p has qb in [4p, 4p+4)
        par = h % 2
        qbl = qb % 4
        si = AVC[0] % 2
        wait("pe", s_act, rec[("exp", (p, h, qb))])
        veng, vval = rec[("cp", 16 + qb)]
        wait("pe", s_dve, vval)  # v copies tb<=qb done (DVE in-order)
        if AVC[0] >= 2:
            # bank si is written next; its previous reader chain (AVC-2) must
            # be done — PSUM banks fault on concurrent PE-write + engine-read
            wait("pe", s_dve, rec[("norm", AVC[0] - 2)])
        for jb in range(qb + 1):
            st, sp_ = jb == 0, jb == qb
            fn = (lambda par=par, jb=jb, qb=qb, h=h, si=si, st=st, sp_=sp_: lambda e: e.matmul(
                ps_av[si][:, 0:65],
                pt_sb[:, par, jb, 128 * qb:128 * qb + 128],
                va_sb[:, jb, h, 0:65],
                start=st, stop=sp_))()
            op("pe", fn, (s_pe, 1) if sp_ else None)
        cnt["pe"] += 1
        rec[("av", AVC[0])] = cnt["pe"]
        rec[("avref", (p, h, qb))] = AVC[0]

        wait("dve", s_pe, rec[("av", AVC[0])])
        fn = (lambda p=p, qbl=qbl, h=h, si=si: lambda e: e.reciprocal(
            rs_sb[:, p, qbl, h:h + 1], ps_av[si][:, 64:65]))()
        op("dve", fn, (s_dve, 1))
        cnt["dve"] += 1
        # stage the AV psum to SBUF, then normalize with a per-partition
        # scalar entirely on DVE (tensor_scalar with PSUM in0 + AP scalar is
        # broken on HW; the SBUF-in0 form is exact)
        fn = (lambda si=si, sj=AVC[0] % 4: lambda e: e.tensor_copy(
            avc_sb[:, sj, :], ps_av[si][:, 0:64]))()
        op("dve", fn, (s_dve, 1))
        cnt["dve"] += 1
        rec[("norm", AVC[0])] = cnt["dve"]  # frees the ps_av slot
        fn = (lambda p=p, qbl=qbl, h=h, sj=AVC[0] % 4: lambda e: e.tensor_scalar_mul(
            atq_sb[:, p, qbl, 64 * h:64 * h + 64], avc_sb[:, sj, :],
            rs_sb[:, p, qbl, h:h + 1]))()
        op("dve", fn, (s_dve, 1))
        cnt["dve"] += 1
        rec[("normref", (p, h, qb))] = cnt["dve"]
        AVC[0] += 1

    def plan_tr(p, hb, qb):
        qbl = qb % 4
        si = TRC[0] % 4
        wait("pe", s_dve, rec[("normref", (p, 2 * hb + 1, qb))])
        if TRC[0] >= 1:
            wait("pe", s_dve, rec[("atcp", TRC[0] - 1)])
        if TRC[0] == 0:
            wait("pe", s_d, 32)  # idb loaded
        fn = (lambda p=p, qbl=qbl, hb=hb, si=si: lambda e: e.matmul(
            ps_tr[:, si, :], atq_sb[:, p, qbl, 128 * hb:128 * hb + 128], idb_sb[:, :],
            is_transpose=True))()
        op("pe", fn, (s_pe, 1))
        cnt["pe"] += 1
        rec[("tr", TRC[0])] = cnt["pe"]

        wait("dve", s_pe, rec[("tr", TRC[0])])
        fn = (lambda hb=hb, qb=qb, si=si: lambda e: e.tensor_copy(
            at_sb[:, hb, 128 * qb:128 * qb + 128], ps_tr[:, si, :]))()
        op("dve", fn, (s_dve, 1))
        cnt["dve"] += 1
        rec[("atcp", TRC[0])] = cnt["dve"]
        rec[("atcp2", (p, hb, qbl))] = cnt["dve"]
        TRC[0] += 1

    def plan_head(p, h, skip_tiles=0):
        """Emit this head's logit tiles + AVs, interleaving the previous
        pair's transposes (on even h >= 2)."""
        ntiles = 4 if p == 0 else 8
        trs = [("tr", h // 2 - 1, 4 * p + q) for q in range(4)] if (h % 2 == 0 and h >= 2) else []
        stream = []
        if p == 0:
            order = ["lg0", "lg1", "av0", "lg2", "av1", "lg3", "av2", "av3"]
        else:
            order = ["lg0", "lg1", "lg2", "lg3", "lg4", "lg5", "av4",
                     "lg6", "av5", "lg7", "av6", "av7"]
        ti = 0
        for it in order:
            kind, idx = it[:2], int(it[2])
            if kind == "lg" and idx < skip_tiles:
                continue
            stream.append((kind, idx))
            if ti < len(trs) and kind == "lg":
                stream.append(trs[ti])
                ti += 1
        for t in trs[ti:]:
            stream.append(t)
        for item in stream:
            if item[0] == "lg":
                plan_lg(p, h, item[1])
            elif item[0] == "av":
                plan_av(p, h, item[1])
            else:
                plan_tr(p, item[1], item[2])

    def plan_oproj(p, interleave_lg=None):
        # trailing pair (hb3) transposes first
        for q in range(4):
            plan_tr(p, 3, 4 * p + q)
        ilg = list(interleave_lg or [])
        for j in range(8):
            tb, mc = 4 * p + j // 2, j % 2
            jj = 8 * p + j
            slot = ps_mm[j % 2]
            if jj == 0:
                wait("pe", s_wo, 16)
            if j >= 2:
                wait("pe", s_dve, rec[("stage", jj - 2)])
            elif p == 0:
                ceng, cval = rec[("cp", 22 + j)]
                wait("pe", s_dve, cval)  # last v copies freed ps_mm
            else:
                wait("pe", s_dve, rec[("stage", 6 + j)])  # phase-0 tail stages
            for nb in range(4):
                st, sp_ = nb == 0, nb == 3
                if nb == 3 or j < 2:
                    wait("pe", s_dve, rec[("atcp2", (p, nb, j // 2))])
                fn = (lambda nb=nb, tb=tb, mc=mc, slot=slot, st=st, sp_=sp_: lambda e: e.matmul(
                    slot[:, :], at_sb[:, nb, tb * 128:(tb + 1) * 128], wo_sb[:, nb, mc * 512:(mc + 1) * 512],
                    start=st, stop=sp_))()
                op("pe", fn, (s_pe, 1) if sp_ else None)
            cnt["pe"] += 1
            rec[("op", jj)] = cnt["pe"]
            if ilg:
                h2, k2 = ilg.pop(0)
                plan_lg(1, h2, k2)

            wait("dve", s_pe, rec[("op", jj)])
            if jj == 0:
                wait("dve", s_d, 48)  # bo loaded
            fn = (lambda jj=jj, mc=mc, slot=slot: lambda e: e.tensor_add(
                stg_own[:, jj, :], slot[:, :], bo_sb[:, mc * 512:(mc + 1) * 512]))()
            op("dve", fn, (s_dve, 1))
            cnt["dve"] += 1
            rec[("stage", jj)] = cnt["dve"]

            wait("sp", s_dve, rec[("stage", jj)])
            pdst = partials[p]
            fn = (lambda j=j, jj=jj, pdst=pdst: lambda e: e.dma_start(
                out=pdst[(j // 2) * 128:(j // 2 + 1) * 128, (j % 2) * 512:(j % 2 + 1) * 512],
                in_=stg_own[:, jj, :]))()
            op("sp", fn, (s_out, 16))

        # fire this phase's ReduceScatter in two 256-row chunks
        for c_ in range(2):
            q_ = 2 * p + c_
            wait("gp", s_out, 16 * (8 * p + 4 * (c_ + 1)))
            op("gp", (lambda p=p, c_=c_, q_=q_: lambda e: e.collective_compute(
                "ReduceScatter", bass.mybir.AluOpType.add,
                replica_groups=[[0, 1], [2, 3], [4, 5], [6, 7]],
                ins=[partials[p][256 * c_:256 * (c_ + 1), :]],
                outs=[red[128 * q_:128 * (q_ + 1), :]]))(), (s_cc, 1))

    stage = os.environ.get("V2STAGE", "full")
    if stage == "lg":
        # logits+exp only, all 8 heads of phase 0
        for h in range(HL):
            for k in range(4):
                plan_lg(0, h, k)
        wait("gp", s_act, cnt["act"])
        for nb in range(4):
            op("gp", (lambda nb=nb: lambda e: e.dma_start(
                out=out[128 * nb:128 * (nb + 1), :], in_=pt_sb[:, 0, nb, :]))(), (s_fin, 16))
        wait("gp", s_fin, 64)
    elif stage == "noav":
        # logits + AV + recip/norm, no transposes
        for h in range(HL):
            if p0 := True:
                for it in ["lg0", "lg1", "av0", "lg2", "av1", "lg3", "av2", "av3"]:
                    kind, idx = it[:2], int(it[2])
                    if kind == "lg":
                        plan_lg(0, h, idx)
                    else:
                        plan_av(0, h, idx)
        wait("gp", s_act, cnt["act"])
        for qbl in range(4):
            op("gp", (lambda qbl=qbl: lambda e: e.dma_start(
                out=out[128 * qbl:128 * (qbl + 1), 0:512],
                in_=atq_sb[:, 0, qbl, :]))(), (s_fin, 16))
        wait("gp", s_fin, 64)
    elif stage == "qkv":
        # truncated: dump qt to out
        wait("gp", s_dve, rec[("cp", 23)][1])
        for nb in range(4):
            op("gp", (lambda nb=nb: lambda e: e.dma_start(
                out=out[128 * nb:128 * (nb + 1), :], in_=qt_sb[:, nb, :]))(), (s_fin, 16))
        wait("gp", s_fin, 64)
    elif stage == "ph0":
        for h in range(HL):
            plan_head(0, h)
        for q in range(4):
            plan_tr(0, 3, q)
        wait("gp", s_dve, rec[("atcp", TRC[0] - 1)])
        for nb in range(4):
            op("gp", (lambda nb=nb: lambda e: e.dma_start(
                out=out[128 * nb:128 * (nb + 1), :], in_=at_sb[:, nb, :]))(), (s_fin, 16))
        wait("gp", s_fin, 64)
    elif stage == "op0":
        for h in range(HL):
            plan_head(0, h)
        plan_oproj(0)
        for q_ in range(2):
            wait("gp", s_cc, q_ + 1)
            op("gp", (lambda q_=q_: lambda e: e.dma_start(
                out=out[128 * q_:128 * (q_ + 1), :],
                in_=red[128 * q_:128 * (q_ + 1), :]))(), (s_fin, 16))
        wait("gp", s_fin, 32)
    else:
        # phase 0
        for h in range(HL):
            plan_head(0, h)
        plan_oproj(0, interleave_lg=[(0, 0), (0, 1), (0, 2), (0, 3)])
        # phase 1 (h0's first 4 logit tiles interleaved into phase-0 O-proj)
        for h in range(HL):
            plan_head(1, h, skip_tiles=4 if h == 0 else 0)
        plan_oproj(1)

        # --- copy RS chunks to out as they land ---
        for q_ in range(4):
            wait("gp", s_cc, q_ + 1)
            op("gp", (lambda q_=q_: lambda e: e.dma_start(
                out=out[128 * q_:128 * (q_ + 1), :],
                in_=red[128 * q_:128 * (q_ + 1), :]))(), (s_fin, 16))
        wait("gp", s_fin, 64)

    _build.ops_debug = {k: list(v) for k, v in ops.items()}

    # ---- emit ----
    def emit(eng, lst):
        for item in lst:
            if item[0] == "wait":
                eng.wait_ge(item[1], item[2])
            else:
                inst = item[1](eng)
                if item[2] is not None:
                    inst.then_inc(item[2][0], item[2][1])

    with nc.Block() as block:
        @block.sync
        def _(e):
            emit(e, ops["sp"])

        @block.tensor
        def _(e):
            emit(e, ops["pe"])

        @block.vector
        def _(e):
            emit(e, ops["dve"])

        @block.scalar
        def _(e):
            emit(e, ops["act"])

        @block.gpsimd
        def _(e):
            emit(e, ops["gp"])

    ctx.close()
    return nc


def _get_nc():
    if "nc" not in _CACHE:
        _CACHE["nc"] = _build()
    return _CACHE["nc"]


def _prep_inputs(x, Wq, Wk, Wv, Wo, bo, rel_pos_bias):
    bf = ml_dtypes.bfloat16
    in_maps = []
    p_idx = np.arange(128)[:, None]
    u_idx = np.arange(T)[None, :]
    for core in range(NCORES):
        b, g = core // 2, core % 2
        xb = np.asarray(x[b], dtype=np.float32)
        xT_h = np.ascontiguousarray(
            xb.T.reshape(8, 128, T).transpose(1, 0, 2)).astype(bf)
        wq_h = np.ascontiguousarray(
            Wq[:, g * NL:(g + 1) * NL].reshape(8, 128, NL).transpose(1, 0, 2)).astype(bf)
        wk_h = np.ascontiguousarray(
            Wk[:, g * NL:(g + 1) * NL].reshape(8, 128, NL).transpose(1, 0, 2)).astype(bf)
        wv_h = np.ascontiguousarray(
            Wv[:, g * NL:(g + 1) * NL].reshape(8, 128, NL).transpose(1, 0, 2)).astype(bf)
        wo_h = np.ascontiguousarray(
            Wo[g * NL:(g + 1) * NL, :].reshape(4, 128, D).transpose(1, 0, 2)).astype(bf)
        db = np.empty((128, HL, T), dtype=bf)
        dif = np.clip(u_idx - p_idx, 0, T - 1)
        msk = u_idx >= p_idx
        for h in range(HL):
            rev = np.asarray(rel_pos_bias[g * HL + h], dtype=np.float32)[::-1]
            db[:, h, :] = np.where(msk, rev[dif], NEG).astype(bf)
        bo_h = np.broadcast_to(np.asarray(bo, np.float32) * 0.5, (128, D)).copy()
        in_maps.append({
            "xT": xT_h, "wq": wq_h, "wk": wk_h, "wv": wv_h, "wo": wo_h,
            "dbias": db, "bo_rep": bo_h, "idb": np.eye(128, dtype=bf),
        })
    return in_maps


def run_on_device(x, Wq, Wk, Wv, Wo, bo, rel_pos_bias, trace=False):
    from concourse.bass_utils import run_bass_kernel_spmd

    nc = _get_nc()
    in_maps = _prep_inputs(x, Wq, Wk, Wv, Wo, bo, rel_pos_bias)
    res = run_bass_kernel_spmd(nc, in_maps, core_ids=list(range(NCORES)), trace=trace)
    outs = []
    for b in range(B):
        ev = res.results[2 * b]["out"]
        od = res.results[2 * b + 1]["out"]
        g = np.empty((T, D), dtype=ev.dtype)
        for blk in range(4):
            g[256 * blk:256 * blk + 128] = ev[128 * blk:128 * blk + 128]
            g[256 * blk + 128:256 * blk + 256] = od[128 * blk:128 * blk + 128]
        outs.append(g)
    out = np.stack(outs).astype(np.float32)
    return out, res


def kernel(x, Wq, Wk, Wv, Wo, bo, rel_pos_bias):
    out, _ = run_on_device(x, Wq, Wk, Wv, Wo, bo, rel_pos_bias, trace=False)
    return out
